# revision 20
# baseline (speedup 1.0000x reference)
"""AxialSpaceTimeTransformer fully fused in ONE Bass program on 8 TRN2 cores.

Sharding (8-way, single chip):
  * t-domain: core c holds frames t in [4c, 4c+4) for both batches.
    Space-attention (over s) and FF are core-local here.
  * s-domain: core c holds spatial positions s in [32c, 32c+32).
    Causal time-attention (over t) is core-local here.

The ENTIRE model runs as a single bass_exec custom call per core:
  pre (rv = rmsnorm(tok) @ vrW)  ->  rv AllToAll (overlapped with L0-2)
  -> 3 space layers -> AllToAll(x, split by batch) -> time layer 3
  -> AllToAll -> 3 space layers -> AllToAll -> time layer 7 -> final norm.
Collectives are in-kernel AllToAlls over internal DRAM bounce buffers,
split by batch half so they overlap with FF/attention compute; x stays
SBUF-resident between layers.

v2 kernel notes vs baseline:
  * scores are computed K-major (lhsT=k, rhs=q) so the exp'd scores are
    already in [kv, q] layout for the AV matmul: no per-head score
    transposes.
  * head pairs share one PSUM bank for scores ([128,512]) and outputs
    ([128,130]); tanh/exp run at 512-wide.
  * (kgamma+1)/softclamp is folded into Wq columns for space layers;
    the per-(token,head) l2norm scale rides the tanh activation's
    per-partition scale operand.
  * softmax denominator via ones-column of v1; per-head output scaling
    (gate/denominator) done with scalar-engine Copy-with-scale-AP.
  * rsqrt Newton iterations on GpSimd (otherwise idle).
"""

import os
import sys
import types

import numpy as np

if "/opt/trn_rl_repo" not in sys.path:
    sys.path.insert(0, "/opt/trn_rl_repo")

# -- antenv.axon_hooks shim (agent image lacks it; bass_utils wants it) --
import antenv  # noqa: E402

if not hasattr(antenv, "axon_hooks"):
    _hooks = types.ModuleType("antenv.axon_hooks")
    _hooks._hook = None
    _hooks.set_axon_ntff_profile_hook = lambda h: setattr(_hooks, "_hook", h)
    _hooks.get_axon_ntff_profile_hook = lambda: _hooks._hook
    sys.modules["antenv.axon_hooks"] = _hooks
    antenv.axon_hooks = _hooks
    try:
        from trn_agent_boot.trn_boot import _ntff_profile_via_ctypes

        _hooks.set_axon_ntff_profile_hook(
            _ntff_profile_via_ctypes("/opt/axon/libaxon_pjrt.so")
        )
    except Exception:
        pass

import jax  # noqa: E402
import jax.numpy as jnp  # noqa: E402
import ml_dtypes  # noqa: E402
from jax.sharding import Mesh, NamedSharding, PartitionSpec as P  # noqa: E402
from jax.experimental.shard_map import shard_map  # noqa: E402

DIM = 768
DEPTH = 8
HEADS = 12
DH = 64
DFF = 2048
SOFTCLAMP = 50.0
B, T, S = 2, 32, 256
EPS = 1e-6
NC = 8
TL = T // NC  # 4 frames/core (t-domain)
SL = S // NC  # 32 positions/core (s-domain)
NTOK = B * TL * S  # 2048 tokens per core in either domain

TIME_LAYERS = (3, 7)


def _make_rotary(n):
    inv = 1.0 / (10000.0 ** (np.arange(0, DH, 2, dtype=np.float32) / DH))
    f = np.arange(n, dtype=np.float32)[:, None] * inv[None, :]
    return np.concatenate([f, f], axis=-1)  # (n, DH)


# ---------------------------------------------------------------------------
# host-side weight packing
# ---------------------------------------------------------------------------


def _pack_weights(inputs):
    """All-layer stacked, norm-folded weights (np arrays)."""
    f32 = np.float32
    bf16 = ml_dtypes.bfloat16
    anw = np.asarray(inputs["attn_norm_w"], f32)[:, :, None]  # (8, 768, 1)
    fnw = np.asarray(inputs["ff_norm_w"], f32)[:, :, None]
    kgam = np.asarray(inputs["k_gamma"], f32)  # (8, 12, 64)
    # (kgam+1)/softclamp: for space layers fold into Wq columns; for time
    # layers it must hit k BEFORE rotary, ship as broadcast row.
    colscale = ((kgam + 1.0) / SOFTCLAMP).reshape(DEPTH, HEADS * DH)
    wq = np.asarray(inputs["Wq"], f32) * anw
    for L in range(DEPTH):
        if L not in TIME_LAYERS:
            wq[L] = wq[L] * colscale[L][None, :]
    g = {}
    g["Wq8"] = wq.astype(bf16)
    g["Wk8"] = (np.asarray(inputs["Wk"], f32) * anw).astype(bf16)
    g["Wv8"] = (np.asarray(inputs["Wv"], f32) * anw).astype(bf16)
    g["Wo8"] = np.asarray(inputs["Wo"], f32).astype(bf16)
    g["Wmg8"] = (
        np.concatenate(
            [
                np.asarray(inputs["Wmix"], f32) * anw,
                np.asarray(inputs["Wg"], f32) * anw,
            ],
            axis=2,
        )
    ).astype(bf16)  # (8, 768, 24)
    g["kg8"] = colscale.astype(f32)  # used by time layers only
    g["Win8"] = (np.asarray(inputs["Win"], f32) * fnw).astype(bf16)
    g["Wout8"] = np.asarray(inputs["Wout"], f32).astype(bf16)
    g["vrW"] = (
        np.asarray(inputs["vr_norm_w"], f32)[:, None]
        * np.asarray(inputs["vr_W"], f32)
    ).astype(bf16)
    # rotary tables for the time layers: partition p = seq*32 + t, t = p % 32
    rot = _make_rotary(T)  # (32, 64), halves identical
    tt = np.tile(np.arange(T), 4)  # (128,) t per partition
    g["rotc"] = np.cos(rot[tt, :]).astype(f32)  # (128, 64)
    g["rotsp"] = np.sin(rot[tt, :32]).astype(f32)  # (128, 32)
    g["rotsn"] = (-np.sin(rot[tt, :32])).astype(f32)  # (128, 32)
    # block-diag causal mask in [kv, q] layout: pk=sq*32+tk, pq=sq'*32+tq
    pk = np.arange(128)
    mask = (pk[:, None] // 32 == pk[None, :] // 32) & (
        pk[:, None] % 32 <= pk[None, :] % 32
    )
    g["maskt"] = np.tile(mask.astype(bf16), (1, 4))  # (128, 512): 4 blocks
    # head-membership mask for feature-major kss: hm[p, kt, h]=1 iff
    # feature (kt*128+p) belongs to head h
    hm = np.zeros((128, KT, HEADS), np.float32)
    for kt in range(KT):
        hm[0:64, kt, 2 * kt] = 1.0
        hm[64:128, kt, 2 * kt + 1] = 1.0
    g["hm"] = hm.astype(bf16)
    return g


# ---------------------------------------------------------------------------
# Bass kernel
# ---------------------------------------------------------------------------
from contextlib import ExitStack  # noqa: E402

import concourse.bacc as bacc  # noqa: E402
import concourse.mybir as mybir  # noqa: E402
import concourse.tile as tile  # noqa: E402
from concourse.bass import ds  # noqa: E402
from concourse.masks import make_identity  # noqa: E402

F32 = mybir.dt.float32
BF16 = mybir.dt.bfloat16
I32 = mybir.dt.int32
AF = mybir.ActivationFunctionType
OP = mybir.AluOpType

NT = 16  # token tiles (2048 tokens)
NSS = 4  # super-seqs of 512 tokens (2 sequences each)
KT = 6  # 768 / 128 feature tiles
H = 12
HP = H // 2  # head pairs

ident_g = {}
ABL = os.environ.get("KV2_ABL", "")


def _emit_rsqrt(nc, eng, pool, out, in_, scale, bias, guard):
    """out = 1/sqrt(max(in_*scale + bias, guard)); quake seed + 3 Newton.

    Runs on `eng` (vector or gpsimd). All tiles SBUF."""
    shp = [128, in_.shape[1]]
    m = pool.tile(shp, F32, name="rs_m", tag="rs_m")
    eng.tensor_scalar(m[:], in_, scale, bias, op0=OP.mult, op1=OP.add)
    eng.tensor_scalar_max(m[:], m[:], guard)
    yi = pool.tile(shp, I32, name="rs_yi", tag="rs_yi")
    eng.tensor_scalar(
        yi[:], m[:].bitcast(I32), 1, None, op0=OP.arith_shift_right
    )
    eng.tensor_scalar(
        yi[:], yi[:], -1, 0x5F3759DF, op0=OP.mult, op1=OP.add
    )
    y = yi[:].bitcast(F32)
    half = pool.tile(shp, F32, name="rs_half", tag="rs_half")
    eng.tensor_scalar_mul(half[:], m[:], 0.5)
    t1 = pool.tile(shp, F32, name="rs_t1", tag="rs_t1")
    for it in range(3):
        eng.tensor_tensor(t1[:], y, y, op=OP.mult)
        eng.tensor_tensor(t1[:], t1[:], half[:], op=OP.mult)
        eng.tensor_scalar(t1[:], t1[:], -1.0, 1.5, op0=OP.mult, op1=OP.add)
        if it < 2:
            eng.tensor_tensor(y, y, t1[:], op=OP.mult)
        else:
            eng.tensor_tensor(out, y, t1[:], op=OP.mult)
    return out


def build_full():
    nc = bacc.Bacc(None, target_bir_lowering=False, num_devices=8)

    x_in = nc.dram_tensor("x_in", [NTOK, DIM], F32, kind="ExternalInput")
    Wq8 = nc.dram_tensor("Wq8", [DEPTH, 768, 768], BF16, kind="ExternalInput")
    Wk8 = nc.dram_tensor("Wk8", [DEPTH, 768, 768], BF16, kind="ExternalInput")
    Wv8 = nc.dram_tensor("Wv8", [DEPTH, 768, 768], BF16, kind="ExternalInput")
    Wo8 = nc.dram_tensor("Wo8", [DEPTH, 768, 768], BF16, kind="ExternalInput")
    Wmg8 = nc.dram_tensor("Wmg8", [DEPTH, 768, 24], BF16, kind="ExternalInput")
    kg8 = nc.dram_tensor("kg8", [DEPTH, 768], F32, kind="ExternalInput")
    Win8 = nc.dram_tensor("Win8", [DEPTH, 768, 4096], BF16, kind="ExternalInput")
    Wout8 = nc.dram_tensor("Wout8", [DEPTH, 2048, 768], BF16, kind="ExternalInput")
    vrW = nc.dram_tensor("vrW", [768, 768], BF16, kind="ExternalInput")
    rotc = nc.dram_tensor("rotc", [128, 64], F32, kind="ExternalInput")
    rotsp = nc.dram_tensor("rotsp", [128, 32], F32, kind="ExternalInput")
    rotsn = nc.dram_tensor("rotsn", [128, 32], F32, kind="ExternalInput")
    maskt = nc.dram_tensor("maskt", [128, 512], BF16, kind="ExternalInput")
    hmask = nc.dram_tensor("hm", [128, KT, H], BF16, kind="ExternalInput")
    x_out = nc.dram_tensor("x_out", [NTOK, DIM], F32, kind="ExternalOutput")

    with tile.TileContext(nc) as tc:
        with ExitStack() as top:
            dram = top.enter_context(tc.tile_pool(name="dram", bufs=1, space="DRAM"))
            # rv bounce: [j, b, tl, sl, d] bf16, one early AllToAll.
            brv_in = dram.tile([NC, B, TL, SL, DIM], BF16, name="brv_in")
            brv_out = dram.tile([NC, B, TL, SL, DIM], BF16, name="brv_out")
            # x bounces, split by batch half for compute overlap.
            bx1_in = [dram.tile([NC, TL, SL, DIM], F32, name=f"bx1_in{b}")
                      for b in range(B)]
            bx1_out = [dram.tile([NC, TL, SL, DIM], F32, name=f"bx1_out{b}")
                       for b in range(B)]
            bx2_in = [dram.tile([NC, SL, TL, DIM], F32, name=f"bx2_in{b}")
                      for b in range(B)]
            bx2_out = [dram.tile([NC, SL, TL, DIM], F32, name=f"bx2_out{b}")
                       for b in range(B)]
            bx3_in = [dram.tile([NC, TL, SL, DIM], F32, name=f"bx3_in{b}")
                      for b in range(B)]
            bx3_out = [dram.tile([NC, TL, SL, DIM], F32, name=f"bx3_out{b}")
                       for b in range(B)]

            const = top.enter_context(tc.tile_pool(name="const", bufs=1))
            xpool = top.enter_context(tc.tile_pool(name="xpool", bufs=1))
            x_sb = xpool.tile([128, NT, 768], F32, name="x_sb")
            nc.sync.dma_start(
                x_sb[:], x_in[:].rearrange("(t p) d -> p t d", p=128)
            )
            ident_f = const.tile([128, 128], F32, name="ident_f")
            make_identity(nc, ident_f)
            ident_b = const.tile([128, 128], BF16, name="ident_b")
            nc.vector.tensor_copy(ident_b[:], ident_f[:])
            ident_g["b"] = ident_b
            rc_sb = const.tile([128, 64], F32, name="rc_sb")
            rsp_sb = const.tile([128, 32], F32, name="rsp_sb")
            rsn_sb = const.tile([128, 32], F32, name="rsn_sb")
            mask_sb = const.tile([128, 512], BF16, name="mask_sb")
            hm_sb = const.tile([128, KT, H], BF16, name="hm_sb")
            nc.sync.dma_start(hm_sb[:], hmask[:])
            nc.sync.dma_start(rc_sb[:], rotc[:])
            nc.sync.dma_start(rsp_sb[:], rotsp[:])
            nc.sync.dma_start(rsn_sb[:], rotsn[:])
            nc.sync.dma_start(mask_sb[:], maskt[:])

            def a2a(src, dst):
                nc.gpsimd.collective_compute(
                    "AllToAll", OP.bypass, replica_groups=[list(range(NC))],
                    ins=[src.opt()], outs=[dst.opt()],
                )

            # ---- pre: rv = rmsnorm(tok) @ vrW -> brv_in, AllToAll early ----
            _pre_rv(nc, tc, x_sb, vrW, brv_in)
            a2a(brv_in, brv_out)

            for L in (0, 1, 2):
                _attn_space(nc, tc, L, x_sb, brv_in, Wq8, Wk8, Wv8, Wo8,
                            Wmg8, hm_sb)
                _ff_layer(nc, tc, L, x_sb, Win8, Wout8)

            # ---- reshard t->s (split by batch), AllToAll, load ----
            for b in range(B):
                _t2s_out_half(nc, x_sb, bx1_in[b], b)
                a2a(bx1_in[b], bx1_out[b])
            _attn_time(nc, tc, 3, x_sb, brv_out, Wq8, Wk8, Wv8, Wo8, Wmg8, kg8,
                       rc_sb, rsp_sb, rsn_sb, mask_sb, bx1_out)
            _ff_layer(nc, tc, 3, x_sb, Win8, Wout8)

            # ---- reshard s->t ----
            for b in range(B):
                _s2t_out_half(nc, x_sb, bx2_in[b], b)
                a2a(bx2_in[b], bx2_out[b])
                _load_t_half(nc, x_sb, bx2_out[b], b)

            for L in (4, 5, 6):
                _attn_space(nc, tc, L, x_sb, brv_in, Wq8, Wk8, Wv8, Wo8,
                            Wmg8, hm_sb)
                _ff_layer(nc, tc, L, x_sb, Win8, Wout8)

            # ---- reshard t->s for layer 7 ----
            for b in range(B):
                _t2s_out_half(nc, x_sb, bx3_in[b], b)
                a2a(bx3_in[b], bx3_out[b])
            _attn_time(nc, tc, 7, x_sb, brv_out, Wq8, Wk8, Wv8, Wo8, Wmg8, kg8,
                       rc_sb, rsp_sb, rsn_sb, mask_sb, bx3_out)
            _ff_layer(nc, tc, 7, x_sb, Win8, Wout8)

            _final_norm(nc, tc, x_sb, x_out)

    nc.compile()
    return nc


# ---------------------------------------------------------------------------
# reshard DMA helpers (all partition APs are contiguous ranges)
# ---------------------------------------------------------------------------


def _t2s_out_half(nc, x_sb, bounce, b):
    """x_sb (t-domain) batch half b -> bounce blocks [j, tl, sl, d]."""
    for j in range(NC):
        src = (
            x_sb[ds(32 * (j % 4), 32), :, :]
            .rearrange("p (bt two) d -> p bt two d", two=2)
            [:, 4 * b : 4 * b + 4, j // 4, :]
        )  # [32(sl), 4(tl), 768]
        dst = bounce[j].rearrange("tl sl d -> sl tl d")  # [32, 4, 768]
        nc.sync.dma_start(dst, src)


def _load_s_half(nc, x_sb, bounce, b):
    """bounce (post-a2a, [c, tl, sl, d]) -> x_sb s-domain tiles of batch b."""
    for q in range(NC):
        g = b * 8 + q
        sl0 = q * 4
        for sq in range(4):
            src = bounce[:, :, sl0 + sq, :]  # [8(c), 4(tl), 768]
            nc.sync.dma_start(x_sb[ds(32 * sq, 32), g, :], src)


def _s2t_out_half(nc, x_sb, bounce, b):
    """x_sb (s-domain) batch half b -> bounce blocks [j, sl, tl, d]."""
    for j in range(NC):
        for sq in range(4):
            src = x_sb[ds(32 * sq + 4 * j, 4), b * 8 : (b + 1) * 8, :]
            # [4(tl), 8(gg), 768]
            dst = bounce[j].rearrange(
                "(gg sq) tl d -> sq tl gg d", sq=4
            )[sq]  # [4(tl), 8(gg), 768]
            nc.sync.dma_start(dst, src)


def _load_t_half(nc, x_sb, bounce, b):
    """bounce (post-a2a, [c, sl, tl, d]) -> x_sb t-domain tiles of batch b."""
    for tl in range(TL):
        for half in range(2):
            tt = b * 8 + 2 * tl + half
            src = bounce[ds(4 * half, 4), :, tl, :]  # [4(c), 32(sl), 768]
            nc.sync.dma_start(x_sb[:, tt, :], src)


# ---------------------------------------------------------------------------
# model phases
# ---------------------------------------------------------------------------


def _rmsnorm_tiles(nc, sp, np_, x_sb, off, nj, pfx, dt=BF16):
    """rmsnorm of nj consecutive token tiles -> tile [128, nj, 768] (dt)."""
    sq = sp.tile([128, 768], F32, name=f"{pfx}sq", tag="sqscr")
    ss = np_.tile([128, nj], F32, name=f"{pfx}ss", tag=f"{pfx}ss")
    for j in range(nj):
        nc.scalar.activation(
            sq[:], x_sb[:, ds(off + j, 1), :].squeeze(1), AF.Square,
            accum_out=ss[:, j : j + 1],
        )
    inv = np_.tile([128, nj], F32, name=f"{pfx}inv", tag=f"{pfx}inv")
    _emit_rsqrt(nc, nc.vector, np_, inv[:], ss[:], 1.0 / 768.0, 1e-6, 1e-30)
    tn_t = sp.tile([128, nj, 768], dt, name=f"{pfx}tn", tag="tokscr")
    for j in range(nj):
        nc.vector.tensor_scalar_mul(
            tn_t[:, j, :], x_sb[:, ds(off + j, 1), :].squeeze(1),
            inv[:, j : j + 1],
        )
    return sq, tn_t


def _transpose_bf(nc, ps_tr, src_t, dst, nj, copy_eng="scalar"):
    """token-major [128, nj, 768] bf16 -> feature-major [128, KT, nj*128]."""
    for kt in range(KT):
        pt = ps_tr.tile([128, nj * 128], BF16, name="pt", tag="ps_trb")
        for j in range(nj):
            nc.tensor.transpose(
                pt[:, j * 128 : (j + 1) * 128],
                src_t[:, j, kt * 128 : (kt + 1) * 128],
                ident_g["b"][:],
            )
        if copy_eng == "scalar":
            nc.scalar.copy(dst[:, kt, :], pt[:])
        else:
            nc.vector.tensor_copy(dst[:, kt, :], pt[:])


def _pre_rv(nc, tc, x_sb, vrW, brv_in):
    with ExitStack() as ctx:
        wp = ctx.enter_context(tc.tile_pool(name="wvr", bufs=1))
        wvr = wp.tile([128, KT, 768], BF16, name="wvr_t")
        nc.sync.dma_start(wvr[:], vrW[:].rearrange("(kt p) m -> p kt m", p=128))
        sp = ctx.enter_context(tc.tile_pool(name="prsp", bufs=2))
        np_ = ctx.enter_context(tc.tile_pool(name="prnp", bufs=2))
        ps_tr = ctx.enter_context(
            tc.tile_pool(name="prps_tr", bufs=2, space="PSUM")
        )
        ps_pj = ctx.enter_context(
            tc.tile_pool(name="prps_pj", bufs=2, space="PSUM")
        )
        for sv in range(NSS):
            b, tlh = sv // 2, sv % 2  # tl pair index
            _, tn_t = _rmsnorm_tiles(nc, sp, np_, x_sb, sv * 4, 4, "pr")
            tn_f = sp.tile([128, KT, 512], BF16, name="prtn_f", tag="prtn_f")
            _transpose_bf(nc, ps_tr, tn_t, tn_f, 4)
            rv_t = sp.tile([128, 4, 768], BF16, name="rv_t", tag="rv_t")
            for j in range(4):
                for nh in range(2):
                    pv = ps_pj.tile([128, 384], F32, name="pv", tag="ps_pj")
                    for kt in range(KT):
                        nc.tensor.matmul(
                            pv[:],
                            lhsT=tn_f[:, kt, j * 128 : (j + 1) * 128],
                            rhs=wvr[:, kt, nh * 384 : (nh + 1) * 384],
                            start=(kt == 0),
                            stop=(kt == KT - 1),
                        )
                    nc.scalar.copy(rv_t[:, j, nh * 384 : (nh + 1) * 384], pv[:])
            for j in range(4):
                tl = 2 * tlh + j // 2
                jj = j % 2
                nc.sync.dma_start(
                    brv_in[ds(4 * jj, 4), b, tl, :, :], rv_t[:, j, :]
                )


def _attn_pools(ctx, tc, pfx, L):
    sp = ctx.enter_context(tc.tile_pool(name=f"{pfx}sp{L}", bufs=2))
    sp2 = ctx.enter_context(tc.tile_pool(name=f"{pfx}sp2{L}", bufs=2))
    hp = ctx.enter_context(tc.tile_pool(name=f"{pfx}hp{L}", bufs=6))
    np_ = ctx.enter_context(tc.tile_pool(name=f"{pfx}np{L}", bufs=4))
    ps_trb = ctx.enter_context(
        tc.tile_pool(name=f"{pfx}ps_trb{L}", bufs=2, space="PSUM")
    )
    ps_pj = ctx.enter_context(
        tc.tile_pool(name=f"{pfx}ps_pj{L}", bufs=2, space="PSUM")
    )
    ps_S = ctx.enter_context(
        tc.tile_pool(name=f"{pfx}ps_S{L}", bufs=1, space="PSUM")
    )
    ps_O = ctx.enter_context(
        tc.tile_pool(name=f"{pfx}ps_O{L}", bufs=2, space="PSUM")
    )
    return sp, sp2, hp, np_, ps_trb, ps_pj, ps_S, ps_O


def _proj_token_major(nc, ps_pj, tn_f, w_t, out_t, nj, copy_eng="scalar",
                      dst_view=None):
    """out[tok, 768] = tn @ W for nj token tiles (token-major output)."""
    for j in range(nj):
        for nh in range(2):
            pk = ps_pj.tile([128, 512], F32, name="pk", tag="ps_pj")
            for kt in range(KT):
                nc.tensor.matmul(
                    pk[:, :384],
                    lhsT=tn_f[:, kt, j * 128 : (j + 1) * 128],
                    rhs=w_t[:, kt, nh * 384 : (nh + 1) * 384],
                    start=(kt == 0),
                    stop=(kt == KT - 1),
                )
            dst = out_t[:, j, nh * 384 : (nh + 1) * 384]
            if copy_eng == "scalar":
                nc.scalar.copy(dst, pk[:, :384])
            else:
                nc.vector.tensor_copy(dst, pk[:, :384])


def _mix_gates(nc, ps_O, np_, tn_f, wmg, nj):
    """sigmoid(tn @ [Wmix|Wg]) -> mgs [128, nj, 24] (mix 0:12, gates 12:24)."""
    mgs = np_.tile([128, nj, 24], F32, name="mgs", tag="mgs")
    for j in range(nj):
        pm = ps_O.tile([128, 136], F32, name="pm", tag="ps_O")
        for kt in range(KT):
            nc.tensor.matmul(
                pm[:, :24],
                lhsT=tn_f[:, kt, j * 128 : (j + 1) * 128],
                rhs=wmg[:, kt, :],
                start=(kt == 0),
                stop=(kt == KT - 1),
            )
        nc.scalar.activation(mgs[:, j, :], pm[:, :24], AF.Tanh, scale=0.5)
    nc.vector.tensor_scalar(
        mgs[:], mgs[:], 0.5, 0.5, op0=OP.mult, op1=OP.add
    )
    return mgs


def _k_l2norm_stats(nc, np_, sp, kraw, nj, pfx):
    """per-(token, head) 1/||k_h||: kinv [128, nj, 12] f32 (SBUF)."""
    kss = np_.tile([128, nj * 12], F32, name=f"{pfx}kss", tag=f"{pfx}kss")
    sqk = sp.tile([128, 768], F32, name=f"{pfx}sqk", tag="sqscr")
    for j in range(nj):
        nc.scalar.activation(sqk[:], kraw[:, j, :], AF.Square)
        nc.vector.tensor_reduce(
            out=kss[:, j * 12 : (j + 1) * 12],
            in_=sqk[:].rearrange("p (h d) -> p h d", h=H),
            axis=mybir.AxisListType.X,
            op=OP.add,
        )
    kinv = np_.tile([128, nj, 12], F32, name=f"{pfx}kinv", tag=f"{pfx}kinv")
    _emit_rsqrt(nc, nc.vector, np_, kinv[:].rearrange("p a b -> p (a b)"),
                kss[:], 1.0 / (SOFTCLAMP * SOFTCLAMP), 0.0, 1e-30)
    return kinv


def _v_lerp(nc, ps_pj, sp, sp2, tn_f, wv, rv_sl, mgs, nj):
    """v1[128, nj, H, 65] bf16: v + mix*(rv-v), ones in col 64."""
    v1 = sp2.tile([128, nj, H, 65], BF16, name="v1", tag="v1")
    for j in range(nj):
        for nh in range(2):
            pv = ps_pj.tile([128, 512], F32, name="pv", tag="ps_pj")
            for kt in range(KT):
                nc.tensor.matmul(
                    pv[:, :384],
                    lhsT=tn_f[:, kt, j * 128 : (j + 1) * 128],
                    rhs=wv[:, kt, nh * 384 : (nh + 1) * 384],
                    start=(kt == 0),
                    stop=(kt == KT - 1),
                )
            vb = sp.tile([128, 384], BF16, name="vb", tag="vb")
            nc.scalar.copy(vb[:], pv[:, :384])
            d = sp.tile([128, 384], BF16, name="dls", tag="dls")
            nc.vector.tensor_tensor(
                d[:], rv_sl[:, j, nh * 384 : (nh + 1) * 384], vb[:],
                op=OP.subtract,
            )
            for h6 in range(6):
                h = 6 * nh + h6
                nc.vector.scalar_tensor_tensor(
                    v1[:, j, h, 0:64],
                    d[:, h6 * 64 : (h6 + 1) * 64],
                    mgs[:, j, h : h + 1],
                    vb[:, h6 * 64 : (h6 + 1) * 64],
                    op0=OP.mult,
                    op1=OP.add,
                )
        nc.vector.memset(v1[:, j, :, 64:65], 1.0)
    return v1


def _attn_epilogue(nc, np_, o_t, pO, mgs, m, jq, q_rel):
    """per (head-pair, q-tile): o = pO[:, :64] * gate / denom."""
    rec2 = np_.tile([128, 2], F32, name="rec2", tag="rec2")
    den = pO[:].rearrange("p (g c) -> p g c", g=2)[:, :, 64]
    nc.vector.reciprocal(rec2[:], den)
    nc.vector.tensor_tensor(
        rec2[:], rec2[:], mgs[:, jq, 12 + 2 * m : 14 + 2 * m], op=OP.mult
    )
    nc.vector.tensor_scalar_mul(
        o_t[:, q_rel, 64 * 2 * m : 64 * 2 * m + 64], pO[:, 0:64],
        rec2[:, 0:1],
    )
    nc.vector.tensor_scalar_mul(
        o_t[:, q_rel, 64 * (2 * m + 1) : 64 * (2 * m + 1) + 64],
        pO[:, 68:132], rec2[:, 1:2],
    )


def _attn_space(nc, tc, L, x_sb, brv_in, Wq8, Wk8, Wv8, Wo8, Wmg8, hm_sb):
    with ExitStack() as ctx:
        wp = ctx.enter_context(tc.tile_pool(name=f"wq{L}", bufs=1))
        wq = wp.tile([128, KT, 768], BF16, name=f"wq_t{L}")
        wk = wp.tile([128, KT, 768], BF16, name=f"wk_t{L}")
        wv = wp.tile([128, KT, 768], BF16, name=f"wv_t{L}")
        wo = wp.tile([128, KT, 768], BF16, name=f"wo_t{L}")
        wmg = wp.tile([128, KT, 24], BF16, name=f"wmg_t{L}")
        for w_t, W in ((wq, Wq8), (wk, Wk8), (wv, Wv8), (wo, Wo8), (wmg, Wmg8)):
            nc.sync.dma_start(
                w_t[:], W[L].rearrange("(kt p) m -> p kt m", p=128)
            )

        (sp, sp2, hp, np_, ps_trb, ps_pj, ps_S, ps_O) = _attn_pools(
            ctx, tc, "s", L
        )

        def seq_body(sv):
            off = sv * 4
            b, tlh = sv // 2, sv % 2
            # ---- rv slice for this super-seq (t-domain blocks, bf16)
            rv_sl = sp.tile([128, 4, 768], BF16, name="rv_sl", tag="rv_sl")
            for j in range(4):
                tl = 2 * tlh + j // 2
                jj = j % 2
                nc.sync.dma_start(
                    rv_sl[:, j, :], brv_in[ds(4 * jj, 4), b, tl, :, :]
                )
            # ---- rmsnorm -> bf16 tn, feature-major tn_f
            _, tn_t = _rmsnorm_tiles(nc, sp, np_, x_sb, off, 4, "a")
            tn_f = sp.tile([128, KT, 512], BF16, name="tn_f", tag="tn_f")
            _transpose_bf(nc, ps_trb, tn_t, tn_f, 4)
            # ---- q projection (feature-major, kgamma/softclamp folded in)
            q_f = sp2.tile([128, KT, 512], BF16, name="q_f", tag="q_f")
            for m in range(KT):
                pq = ps_pj.tile([128, 512], F32, name="pq", tag="ps_pj")
                for kt in range(KT):
                    nc.tensor.matmul(
                        pq[:],
                        lhsT=wq[:, kt, m * 128 : (m + 1) * 128],
                        rhs=tn_f[:, kt, :],
                        start=(kt == 0),
                        stop=(kt == KT - 1),
                    )
                nc.vector.tensor_copy(q_f[:, m, :], pq[:])
            # ---- k projection (feature-major) + per-(token,head) 50/||k||
            k_f = sp2.tile([128, KT, 512], BF16, name="k_f", tag="k_f")
            for m in range(KT):
                pk = ps_pj.tile([128, 512], F32, name="pk", tag="ps_pj")
                for kt in range(KT):
                    nc.tensor.matmul(
                        pk[:],
                        lhsT=wk[:, kt, m * 128 : (m + 1) * 128],
                        rhs=tn_f[:, kt, :],
                        start=(kt == 0),
                        stop=(kt == KT - 1),
                    )
                nc.scalar.copy(k_f[:, m, :], pk[:])
            k2 = sp.tile([128, KT, 512], BF16, name="k2", tag="tokscr")
            nc.vector.tensor_tensor(k2[:], k_f[:], k_f[:], op=OP.mult)
            kss = np_.tile([128, 4, 12], F32, name="skss", tag="skss")
            for j in range(4):
                pks = ps_O.tile([128, 136], F32, name="pks", tag="ps_O")
                for kt in range(KT):
                    nc.tensor.matmul(
                        pks[:, :12],
                        lhsT=k2[:, kt, j * 128 : (j + 1) * 128],
                        rhs=hm_sb[:, kt, :],
                        start=(kt == 0),
                        stop=(kt == KT - 1),
                    )
                nc.scalar.copy(kss[:, j, :], pks[:, :12])
            kinv = np_.tile([128, 4, 12], F32, name="skinv", tag="skinv")
            _emit_rsqrt(nc, nc.vector, np_,
                        kinv[:].rearrange("p a b -> p (a b)"),
                        kss[:].rearrange("p a b -> p (a b)"),
                        1.0 / (SOFTCLAMP * SOFTCLAMP), 0.0, 1e-30)
            # ---- mix / gates
            mgs = _mix_gates(nc, ps_O, np_, tn_f, wmg, 4)
            # ---- v projection + value-residual lerp
            v1 = _v_lerp(nc, ps_pj, sp, sp2, tn_f, wv, rv_sl, mgs, 4)
            # ---- attention per (sequence, head-pair); scores K-major
            o_t = sp.tile([128, 4, 768], BF16, name="o_t", tag="o_t")
            for s_loc in range(2):
                qcols = ds(s_loc * 256, 256)
                for m in range(HP):
                    pts = []
                    for kvt in range(2):
                        j = 2 * s_loc + kvt
                        pSs = [
                            ps_S.tile([128, 256], F32, name="pSa", tag="ps_Sa"),
                            ps_S.tile([128, 256], F32, name="pSb", tag="ps_Sb"),
                        ]
                        for par in range(2):
                            po = 64 * par
                            nc.tensor.matmul(
                                pSs[par][:],
                                lhsT=k_f[po : po + 64, m,
                                         j * 128 : (j + 1) * 128],
                                rhs=q_f[po : po + 64, m, qcols],
                                start=True,
                                stop=True,
                            )
                        pt_b = hp.tile([128, 512], BF16, name="pt_b",
                                       tag="pt_b")
                        for par in range(2):
                            nc.scalar.activation(
                                pt_b[:, par * 256 : (par + 1) * 256],
                                pSs[par][:],
                                AF.Exp,
                                scale=kinv[:, j, 2 * m + par : 2 * m + par + 1],
                            )
                        pts.append(pt_b)
                    for qt in range(2):
                        pO = ps_O.tile([128, 136], F32, name="pO", tag="ps_O")
                        for par in range(2):
                            for kvt in range(2):
                                nc.tensor.matmul(
                                    pO[:, par * 68 : par * 68 + 65],
                                    lhsT=pts[kvt][
                                        :, par * 256 + qt * 128 :
                                        par * 256 + (qt + 1) * 128],
                                    rhs=v1[:, 2 * s_loc + kvt, 2 * m + par, :],
                                    start=(kvt == 0),
                                    stop=(kvt == 1),
                                )
                        _attn_epilogue(nc, np_, o_t, pO, mgs, m,
                                       2 * s_loc + qt, 2 * s_loc + qt)
            # ---- transpose o -> o_f, then Wo and residual add
            o_f = sp.tile([128, KT, 512], BF16, name="o_f", tag="tokscr")
            _transpose_bf(nc, ps_trb, o_t, o_f, 4, copy_eng="vector")
            for j in range(4):
                for nh in range(2):
                    px = ps_pj.tile([128, 512], F32, name="px", tag="ps_pj")
                    for kt in range(KT):
                        nc.tensor.matmul(
                            px[:, :384],
                            lhsT=o_f[:, kt, j * 128 : (j + 1) * 128],
                            rhs=wo[:, kt, nh * 384 : (nh + 1) * 384],
                            start=(kt == 0),
                            stop=(kt == KT - 1),
                        )
                    xs = x_sb[:, ds(off + j, 1), nh * 384 : (nh + 1) * 384]
                    xs = xs.squeeze(1)
                    nc.vector.tensor_tensor(xs, xs, px[:, :384], op=OP.add)

        for _sv in range(NSS):
            seq_body(_sv)


def _attn_time(nc, tc, L, x_sb, brv_out, Wq8, Wk8, Wv8, Wo8, Wmg8, kg8,
               rc_sb, rsp_sb, rsn_sb, mask_sb, bx_out):
    with ExitStack() as ctx:
        wp = ctx.enter_context(tc.tile_pool(name=f"twq{L}", bufs=1))
        wq = wp.tile([128, KT, 768], BF16, name=f"twq_t{L}")
        wk = wp.tile([128, KT, 768], BF16, name=f"twk_t{L}")
        wv = wp.tile([128, KT, 768], BF16, name=f"twv_t{L}")
        wo = wp.tile([128, KT, 768], BF16, name=f"two_t{L}")
        wmg = wp.tile([128, KT, 24], BF16, name=f"twmg_t{L}")
        kgbc = wp.tile([128, 768], F32, name=f"tkgbc{L}")
        for w_t, W in ((wq, Wq8), (wk, Wk8), (wv, Wv8), (wo, Wo8), (wmg, Wmg8)):
            nc.sync.dma_start(
                w_t[:], W[L].rearrange("(kt p) m -> p kt m", p=128)
            )
        nc.sync.dma_start(kgbc[:], kg8[L : L + 1, :].partition_broadcast(128))

        (sp, sp2, hp, np_, ps_trb, ps_pj, ps_S, ps_O) = _attn_pools(
            ctx, tc, "t", L
        )

        rc_bc = rc_sb[:].unsqueeze(1).broadcast_to([128, H, 64])
        rsp_bc = rsp_sb[:].unsqueeze(1).broadcast_to([128, H, 32])
        rsn_bc = rsn_sb[:].unsqueeze(1).broadcast_to([128, H, 32])

        def _rotary(x_r, rtmp, rtmp2):
            """x_r [128, 768] bf16 view; rotate in place."""
            xv = x_r.rearrange("p (h half d) -> p h half d", h=H, half=2)
            tv = rtmp[:].rearrange("p (h half d) -> p h half d", h=H, half=2)
            nc.vector.tensor_tensor(tv[:, :, 0, :], xv[:, :, 1, :], rsn_bc,
                                    op=OP.mult)
            nc.vector.tensor_tensor(tv[:, :, 1, :], xv[:, :, 0, :], rsp_bc,
                                    op=OP.mult)
            nc.vector.tensor_tensor(
                rtmp2[:].rearrange("p (h d) -> p h d", h=H),
                x_r.rearrange("p (h d) -> p h d", h=H), rc_bc, op=OP.mult
            )
            nc.vector.tensor_tensor(x_r, rtmp2[:], rtmp[:], op=OP.add)

        def seq_body(sv):
            off = sv * 4
            b = sv // 2
            # ---- load x for these token tiles (first time-layer reshard)
            if bx_out is not None:
                q0 = (sv % 2) * 4
                for q in range(q0, q0 + 4):
                    g = b * 8 + q
                    sl0 = q * 4
                    for sq in range(4):
                        nc.sync.dma_start(
                            x_sb[ds(32 * sq, 32), g, :],
                            bx_out[b][:, :, sl0 + sq, :],
                        )
            # ---- rv slice (s-domain blocks from brv_out, bf16)
            rv_sl = sp.tile([128, 4, 768], BF16, name="trv_sl", tag="trv_sl")
            for j in range(4):
                sl0 = ((off + j) % 8) * 4
                for sq_i in range(4):
                    nc.sync.dma_start(
                        rv_sl[ds(32 * sq_i, 32), j, :],
                        brv_out[:, b, :, sl0 + sq_i, :],
                    )
            # ---- rmsnorm -> bf16 tn, feature-major tn_f
            _, tn_t = _rmsnorm_tiles(nc, sp, np_, x_sb, off, 4, "t")
            tn_f = sp.tile([128, KT, 512], BF16, name="ttn_f", tag="ttn_f")
            _transpose_bf(nc, ps_trb, tn_t, tn_f, 4)
            rtmp = sp.tile([128, 768], BF16, name="rtmp", tag="rtmp")
            rtmp2 = sp.tile([128, 768], BF16, name="rtmp2", tag="rtmp2")
            # ---- q projection token-major + rotary -> transpose to q_f
            qraw = sp.tile([128, 4, 768], BF16, name="qraw", tag="tokscr")
            _proj_token_major(nc, ps_pj, tn_f, wq, qraw, 4)
            for j in range(4):
                _rotary(qraw[:, j, :], rtmp, rtmp2)
            q_f = sp2.tile([128, KT, 512], BF16, name="tq_f", tag="tq_f")
            _transpose_bf(nc, ps_trb, qraw, q_f, 4)
            # ---- k projection + kgamma scale + rotary -> k_f
            kraw = sp.tile([128, 4, 768], BF16, name="tkraw", tag="tokscr")
            _proj_token_major(nc, ps_pj, tn_f, wk, kraw, 4)
            kinv = _k_l2norm_stats(nc, np_, sp, kraw, 4, "t")
            for j in range(4):
                nc.vector.tensor_tensor(
                    kraw[:, j, :], kraw[:, j, :], kgbc[:], op=OP.mult
                )
                _rotary(kraw[:, j, :], rtmp, rtmp2)
            k_f = sp2.tile([128, KT, 512], BF16, name="tk_f", tag="tk_f")
            _transpose_bf(nc, ps_trb, kraw, k_f, 4)
            # ---- mix / gates
            mgs = _mix_gates(nc, ps_O, np_, tn_f, wmg, 4)
            # ---- v projection + value-residual lerp
            v1 = _v_lerp(nc, ps_pj, sp, sp2, tn_f, wv, rv_sl, mgs, 4)
            # ---- attention per (seq, head-pair): block-local causal
            o_t = sp.tile([128, 4, 768], BF16, name="to_t", tag="to_t")
            for s_loc in range(2):
                for m in range(HP):
                    # split score tiles per parity: same-row-group matmuls
                    # may share a bank, different row groups may not.
                    pSs = [
                        ps_S.tile([128, 256], F32, name="tpSa", tag="ps_Sa"),
                        ps_S.tile([128, 256], F32, name="tpSb", tag="ps_Sb"),
                    ]
                    for par in range(2):
                        po = 64 * par
                        for tt in range(2):
                            j = 2 * s_loc + tt
                            nc.tensor.matmul(
                                pSs[par][:, tt * 128 : (tt + 1) * 128],
                                lhsT=k_f[po : po + 64, m,
                                         j * 128 : (j + 1) * 128],
                                rhs=q_f[po : po + 64, m,
                                        j * 128 : (j + 1) * 128],
                                start=True,
                                stop=True,
                            )
                    pt_b = hp.tile([128, 512], BF16, name="tpt_b", tag="tpt_b")
                    for par in range(2):
                        for tt in range(2):
                            j = 2 * s_loc + tt
                            nc.scalar.activation(
                                pt_b[:, (par * 2 + tt) * 128 :
                                     (par * 2 + tt + 1) * 128],
                                pSs[par][:, tt * 128 : (tt + 1) * 128],
                                AF.Exp,
                                scale=kinv[:, j, 2 * m + par : 2 * m + par + 1],
                            )
                    nc.gpsimd.tensor_tensor(
                        pt_b[:], pt_b[:], mask_sb[:], op=OP.mult
                    )
                    for tt in range(2):
                        pO = ps_O.tile([128, 136], F32, name="tpO", tag="ps_O")
                        for par in range(2):
                            nc.tensor.matmul(
                                pO[:, par * 68 : par * 68 + 65],
                                lhsT=pt_b[:, (par * 2 + tt) * 128 :
                                          (par * 2 + tt + 1) * 128],
                                rhs=v1[:, 2 * s_loc + tt, 2 * m + par, :],
                                start=True,
                                stop=True,
                            )
                        _attn_epilogue(nc, np_, o_t, pO, mgs, m,
                                       2 * s_loc + tt, 2 * s_loc + tt)
            # ---- transpose o -> o_f, then Wo and residual add
            o_f = sp.tile([128, KT, 512], BF16, name="to_f", tag="tokscr")
            _transpose_bf(nc, ps_trb, o_t, o_f, 4, copy_eng="vector")
            for j in range(4):
                for nh in range(2):
                    px = ps_pj.tile([128, 512], F32, name="tpx", tag="ps_pj")
                    for kt in range(KT):
                        nc.tensor.matmul(
                            px[:, :384],
                            lhsT=o_f[:, kt, j * 128 : (j + 1) * 128],
                            rhs=wo[:, kt, nh * 384 : (nh + 1) * 384],
                            start=(kt == 0),
                            stop=(kt == KT - 1),
                        )
                    xs = x_sb[:, ds(off + j, 1), nh * 384 : (nh + 1) * 384]
                    xs = xs.squeeze(1)
                    nc.vector.tensor_tensor(xs, xs, px[:, :384], op=OP.add)

        for _sv in range(NSS):
            seq_body(_sv)


def _ff_layer(nc, tc, L, x_sb, Win8, Wout8):
    with ExitStack() as ctx:
        wop = ctx.enter_context(tc.tile_pool(name=f"wop{L}", bufs=1))
        win = wop.tile([128, KT, 4096], BF16, name=f"win_t{L}")
        nc.sync.dma_start(
            win[:], Win8[L].rearrange("(kt p) m -> p kt m", p=128)
        )
        wout = wop.tile([128, 16, 768], BF16, name=f"wout_t{L}")
        nc.sync.dma_start(
            wout[:], Wout8[L].rearrange("(kt p) m -> p kt m", p=128)
        )
        sp = ctx.enter_context(tc.tile_pool(name=f"fsp{L}", bufs=2))
        up = ctx.enter_context(tc.tile_pool(name=f"fup{L}", bufs=2))
        np_ = ctx.enter_context(tc.tile_pool(name=f"fnp{L}", bufs=2))
        ps_tr = ctx.enter_context(
            tc.tile_pool(name=f"fps_tr{L}", bufs=2, space="PSUM")
        )
        ps_h = ctx.enter_context(
            tc.tile_pool(name=f"fps_h{L}", bufs=4, space="PSUM")
        )
        ps_xd = ctx.enter_context(
            tc.tile_pool(name=f"fps_xd{L}", bufs=2, space="PSUM")
        )

        def chunk_body(cv):
            coff = cv * 4
            _, tn2 = _rmsnorm_tiles(nc, sp, np_, x_sb, coff, 4, "f")
            tn2f = sp.tile([128, KT, 512], BF16, name="tn2f", tag="tn2f")
            for kt in range(KT):
                pt = ps_tr.tile([128, 512], BF16, name="fpt", tag="fps_tr")
                for j in range(4):
                    nc.tensor.transpose(
                        pt[:, j * 128 : (j + 1) * 128],
                        tn2[:, j, kt * 128 : (kt + 1) * 128],
                        ident_g["b"][:],
                    )
                nc.scalar.copy(tn2f[:, kt, :], pt[:])
            # ---- h = tn2 @ Win; u = a * gelu(g)
            u = up.tile([128, 16, 512], BF16, name="u", tag="u")
            gl = sp.tile([128, 512], F32, name="gl", tag="gl")
            for m in range(16):
                pa = ps_h.tile([128, 512], F32, name="pa", tag="fps_h")
                pg = ps_h.tile([128, 512], F32, name="pg", tag="fps_h")
                for kt in range(KT):
                    nc.tensor.matmul(
                        pa[:], lhsT=win[:, kt, m * 128 : (m + 1) * 128],
                        rhs=tn2f[:, kt, :],
                        start=(kt == 0), stop=(kt == KT - 1),
                    )
                for kt in range(KT):
                    nc.tensor.matmul(
                        pg[:],
                        lhsT=win[:, kt, 2048 + m * 128 : 2048 + (m + 1) * 128],
                        rhs=tn2f[:, kt, :],
                        start=(kt == 0), stop=(kt == KT - 1),
                    )
                nc.scalar.activation(gl[:], pg[:], AF.Gelu)
                nc.vector.tensor_tensor(u[:, m, :], pa[:], gl[:], op=OP.mult)
            # ---- x += u @ Wout
            for j in range(4):
                for nh in range(2):
                    px = ps_xd.tile([128, 384], F32, name="fpx", tag="fps_xd")
                    for ktf in range(16):
                        nc.tensor.matmul(
                            px[:],
                            lhsT=u[:, ktf, j * 128 : (j + 1) * 128],
                            rhs=wout[:, ktf, nh * 384 : (nh + 1) * 384],
                            start=(ktf == 0),
                            stop=(ktf == 15),
                        )
                    xs = x_sb[:, ds(coff + j, 1), nh * 384 : (nh + 1) * 384]
                    xs = xs.squeeze(1)
                    nc.vector.tensor_tensor(xs, xs, px[:], op=OP.add)

        for _cv in range(4):
            chunk_body(_cv)


def _final_norm(nc, tc, x_sb, x_out):
    with ExitStack() as ctx:
        sp = ctx.enter_context(tc.tile_pool(name="fnsp", bufs=2))
        np_ = ctx.enter_context(tc.tile_pool(name="fnnp", bufs=2))
        for sv in range(8):
            off = sv * 2
            sq = sp.tile([128, 768], F32, name="fnsq", tag="fnsq")
            ss = np_.tile([128, 2], F32, name="fnss", tag="fnss")
            for j in range(2):
                nc.scalar.activation(
                    sq[:], x_sb[:, ds(off + j, 1), :].squeeze(1), AF.Square,
                    accum_out=ss[:, j : j + 1],
                )
            inv = np_.tile([128, 2], F32, name="fninv", tag="fninv")
            _emit_rsqrt(nc, nc.vector, np_, inv[:], ss[:], 1.0 / 768.0,
                        1e-6, 1e-30)
            out_t = sp.tile([128, 2, 768], F32, name="fnout", tag="fnout")
            for j in range(2):
                nc.vector.tensor_scalar_mul(
                    out_t[:, j, :], x_sb[:, ds(off + j, 1), :].squeeze(1),
                    inv[:, j : j + 1],
                )
            nc.sync.dma_start(
                x_out[ds(sv * 256, 256), :].rearrange("(j p) d -> p j d", p=128),
                out_t[:],
            )


# ---------------------------------------------------------------------------
# binding / execution
# ---------------------------------------------------------------------------


def _collect_io(nc):
    import jax

    in_names, out_names, out_avals = [], [], []
    pname = nc.partition_id_tensor.name if nc.partition_id_tensor else None
    for alloc in nc.m.functions[0].allocations:
        if not isinstance(alloc, mybir.MemoryLocationSet):
            continue
        if not alloc.memorylocations:
            continue
        name = alloc.memorylocations[0].name
        if alloc.kind == "ExternalInput" and name != pname:
            in_names.append(name)
        elif alloc.kind == "ExternalOutput":
            out_names.append(name)
            out_avals.append(
                jax.core.ShapedArray(
                    tuple(alloc.tensor_shape), mybir.dt.np(alloc.dtype)
                )
            )
    return in_names, out_names, out_avals


_PIPE = None


def _tok_to_bt(tokens):
    """(B, T, S, D) -> global (NC*B*TL, S, D): rows (c, b, tl), t = 4c+tl."""
    return np.ascontiguousarray(
        tokens.transpose(1, 0, 2, 3)
        .reshape(NC, TL, B, S, DIM)
        .transpose(0, 2, 1, 3, 4)
    ).reshape(NC * B * TL, S, DIM)


def _out_to_full(out):
    """(NC*B*SL, T, D) rows (c, b, sl) with s=32c+sl -> (B, T, S, D)."""
    out = out.reshape(NC, B, SL, T, DIM).transpose(1, 3, 0, 2, 4)
    return out.reshape(B, T, S, DIM)


def _build_pipeline(inputs):
    devs = jax.devices()[:NC]
    mesh = Mesh(np.asarray(devs), ("core",))
    shard = NamedSharding(mesh, P("core"))

    nc = build_full()
    from concourse import bass2jax
    from concourse.bass2jax import _bass_exec_p

    in_names, out_names, out_avals = _collect_io(nc)
    bind_names = tuple(in_names + out_names)
    pid_name = nc.partition_id_tensor.name if nc.partition_id_tensor else None
    full_names = bind_names + ((pid_name,) if pid_name else ())

    def bass_body(*args):
        ops = list(args)
        if pid_name is not None:
            ops.append(bass2jax.partition_id_tensor())
        outs = _bass_exec_p.bind(
            *ops,
            out_avals=tuple(out_avals),
            in_names=full_names,
            out_names=tuple(out_names),
            lowering_input_output_aliases=(),
            sim_require_finite=True,
            sim_require_nnan=True,
            nc=nc,
        )
        return tuple(outs)

    percore = {"x_in"} | set(out_names)
    in_specs = tuple(P("core") if n in percore else P() for n in bind_names)
    out_specs = (P("core"),) * len(out_names)
    nout = len(out_names)
    donate = tuple(range(len(bind_names) - nout, len(bind_names)))
    if os.environ.get("KV2_SIM", "0") == "1":
        donate = ()
    bass_jit = jax.jit(
        shard_map(bass_body, mesh=mesh, in_specs=in_specs,
                  out_specs=out_specs, check_rep=False),
        donate_argnums=donate,
    )

    pk = _pack_weights(inputs)
    repl = NamedSharding(mesh, P())
    wdev = {k: jax.device_put(v, repl) for k, v in pk.items()}

    zero_outs = jax.jit(
        lambda: tuple(
            jnp.zeros((NC * aval.shape[0],) + aval.shape[1:], aval.dtype)
            for aval in out_avals
        ),
        out_shardings=tuple(shard for _ in out_avals),
    )

    def run(tok_bt):
        tok = jax.device_put(tok_bt, shard).reshape(NC * NTOK, DIM)
        ops = []
        for nme in in_names:
            if nme == "x_in":
                ops.append(tok)
            else:
                ops.append(wdev[nme])
        outs = bass_jit(*ops, *zero_outs())
        return dict(zip(out_names, outs))

    run.nc = nc
    run.in_names = in_names
    run.out_names = out_names
    run.weights = pk
    return run


def kernel(**inputs):
    global _PIPE
    tokens = np.asarray(inputs["tokens"], dtype=np.float32)
    tok_bt = _tok_to_bt(tokens)

    if _PIPE is None:
        _PIPE = _build_pipeline(inputs)
    outs = _PIPE(jnp.asarray(tok_bt))
    out = np.asarray(jax.block_until_ready(outs["x_out"]))

    out = _out_to_full(out.reshape(NC * B * SL, T, DIM))
    out = out * np.asarray(inputs["final_norm_w"], np.float32)
    _PIPE.last_outs = outs
    return np.ascontiguousarray(out.astype(np.float32))
